# revision 2
# baseline (speedup 1.0000x reference)
"""Criss-cross (CCNet) attention kernel for Trainium2, 8 NeuronCores — v3.

Sharding: core c in 0..7 -> batch b = c//2, value-channel half h = c%2.

v4 changes vs v3 (223-265 us):
  - constants load via the scalar-engine HWDGE queue so the x stream owns
    the sync queue from t=0; x prefetches 4 chunks deep.
  - K2T: an x-major copy of k, so column-pass LDWEIGHTS are contiguous
    (strided weight loads measured 1.5-2.7x slower and starve the PE).
  - column pass emits energy matmuls interleaved between aggregation
    groups so the PE queue never head-of-line blocks on a PSUM WAR.

v3 changes vs v2 (232 us):
  - sums S come from two ones-columns appended to each V slot (N=258
    aggregation matmuls) instead of 256 separate N=2 matmuls: fewer, denser
    PE instructions (HAM-friendlier) and no persistent S PSUM bank.
  - the column-pass diagonal mask is a DVE multiply of P by a 0/1 mask
    (bf16 2x mode) instead of two N=512 accumulate-matmuls per superchunk.
  - V-transpose staging buffer prefetches 3 superchunks deep (SWDGE
    latency decoupled), weight/mask constants load before the x stream,
    x prefetches 3 chunks deep, and outputs drain in 1032-column strips.
"""

import numpy as np

import concourse.tile as tile
from concourse import bacc, mybir
from concourse.bass_utils import run_bass_kernel_spmd

B, C, H, W = 4, 512, 128, 128
HW = H * W
CV = 256          # v channels per core
VBW = 258         # V slot width: 256 channels + 2 ones columns
N_CORES = 8

F32 = mybir.dt.float32
F16 = mybir.dt.float16
BF16 = mybir.dt.bfloat16
EXP = mybir.ActivationFunctionType.Exp
COPY = mybir.ActivationFunctionType.Copy
MULT = mybir.AluOpType.mult

_CACHE = {}


def _build(with_bias):
    nc = bacc.Bacc("TRN2", target_bir_lowering=False, debug=False,
                   num_devices=N_CORES)
    nck = 5 if with_bias else 4
    xrows = C + (2 if with_bias else 0)

    xin = nc.dram_tensor("xin", [xrows, HW], F16, kind="ExternalInput").ap()
    wqk = nc.dram_tensor("wqk", [xrows, 128], F16, kind="ExternalInput").ap()
    wv = nc.dram_tensor("wv", [xrows, CV], F16, kind="ExternalInput").ap()
    dmask = nc.dram_tensor("dmask", [128, 512], BF16,
                           kind="ExternalInput").ap()
    onesc = nc.dram_tensor("onesc", [128, 96], BF16, kind="ExternalInput").ap()

    # V staging, x-major: flat row index = x*128 + y
    vscr = nc.dram_tensor("vscr", [HW, CV], BF16).ap()
    # orow: [x, chunk(32), 4*258] ; ocol: [y, half-superchunk(32), 4*258]
    orow = nc.dram_tensor("orow", [128, 32 * 1032], BF16,
                          kind="ExternalOutput").ap()
    ocol = nc.dram_tensor("ocol", [128, 32 * 1032], BF16,
                          kind="ExternalOutput").ap()

    xin4 = xin[0:C, :].rearrange("(k p) n -> p k n", p=128)
    vscr_w = vscr.rearrange("(x g t) c -> x g t c", g=32, t=4)
    vscr_r = vscr.rearrange("(s xi y) c -> y s xi c", s=16, xi=8)
    orow_v = orow.rearrange("p (g n) -> p g n", n=1032)
    ocol_v = ocol.rearrange("p (g n) -> p g n", n=1032)

    with tile.TileContext(nc) as tc:
        with (
            tc.tile_pool(name="cst", bufs=1) as cst,
            tc.tile_pool(name="xs", bufs=4) as xsp,
            tc.tile_pool(name="p4", bufs=4) as p4p,
            tc.tile_pool(name="o16r", bufs=2) as o16rp,
            tc.tile_pool(name="o16c", bufs=3) as o16cp,
            tc.tile_pool(name="wk", bufs=4, space="PSUM") as wkp,
            tc.tile_pool(name="pO", bufs=2, space="PSUM") as pOp,
        ):
            row_state = {}
            col_state = {}
            evq = [0]

            def ev(dst, src):
                # alternate evacuation engine to balance ACT vs DVE
                evq[0] += 1
                if evq[0] % 2 == 0:
                    nc.scalar.activation(dst, src, COPY)
                else:
                    nc.vector.tensor_copy(dst, src)

            # ---- constants first so the first matmul can start early ----
            WQK = cst.tile([128, nck * 128], F16)
            for k in range(nck):
                rows = 128 if k < 4 else 2
                nc.scalar.dma_start(WQK[0:rows, k * 128:k * 128 + 128],
                                    wqk[k * 128:k * 128 + rows, :])
            WV = cst.tile([128, nck * CV], F16)
            for k in range(nck):
                rows = 128 if k < 4 else 2
                nc.scalar.dma_start(WV[0:rows, k * CV:k * CV + CV],
                                    wv[k * 128:k * 128 + rows, :])
            DMASK = cst.tile([128, 512], BF16)
            nc.scalar.dma_start(DMASK[:], dmask[:])

            QK = cst.tile([128, HW], F16)
            K2 = cst.tile([64, HW], F16)
            K2T = cst.tile([64, HW], F16)
            VB = cst.tile([128, 8 * VBW], BF16)
            VTB = cst.tile([128, 32 * VBW], BF16)
            nc.scalar.dma_start(
                VB[:].rearrange("p (s w) -> p s w", w=VBW)[:, :, 256:258],
                onesc[:, 0:16].rearrange("p (s w) -> p s w", w=2))
            nc.scalar.dma_start(
                VTB[:].rearrange("p (s w) -> p s w", w=VBW)[:, :, 256:258],
                onesc[:, 16:80].rearrange("p (s w) -> p s w", w=2))

            def load_x(ch):
                xm = xsp.tile([128, 4 * 512], F16, tag="xm")
                nc.sync.dma_start(
                    xm[:].rearrange("p (k n) -> p k n", n=512),
                    xin4[:, :, ch * 512:(ch + 1) * 512])
                ent = {"xm": xm}
                if with_bias:
                    xb2 = xsp.tile([2, 512], F16, tag="xb2")
                    nc.sync.dma_start(xb2[:],
                                      xin[C:C + 2, ch * 512:(ch + 1) * 512])
                    ent["xb2"] = xb2
                row_state[("xs", ch)] = ent

            for ch in range(4):
                load_x(ch)

            qk_of = QK[0:64, :].rearrange("c (y x) -> c y x", x=128)
            k2_of = K2[:].rearrange("c (y x) -> c y x", x=128)
            k2t_x = K2T[:].rearrange("c (x y) -> c x y", y=128)
            vb_s = VB[:].rearrange("p (s w) -> p s w", w=VBW)
            vtb_s = VTB[:].rearrange("p (s w) -> p s w", w=VBW)

            # =================== row pass ===================
            def row_head(ch):
                st = row_state.pop(("xs", ch))
                xm = st["xm"]
                csl = slice(ch * 512, (ch + 1) * 512)
                vsl = (ch % 2) * 4      # VB slot base

                # qk projection for these 512 pixels
                pqk = wkp.tile([128, 512], F32, tag="wk")
                for k in range(nck):
                    rows = 128 if k < 4 else 2
                    rhs = xm[:, k * 512:(k + 1) * 512] if k < 4 \
                        else st["xb2"][:]
                    nc.tensor.matmul(pqk[:],
                                     WQK[0:rows, k * 128:(k + 1) * 128],
                                     rhs, start=(k == 0), stop=(k == nck - 1))
                ev(QK[:, csl], pqk[:])
                nc.vector.tensor_copy(K2[:, csl], QK[64:128, csl])
                ev(k2t_x[:, :, ch * 4:ch * 4 + 4],
                   QK[64:128, csl].rearrange("c (t x) -> c x t", x=128))

                # v projection, 2 groups of 2 rows -> [128px, 512] PSUM
                for g in range(2):
                    pv = wkp.tile([128, 512], F32, tag="wk")
                    for q2 in range(2):
                        yy = g * 2 + q2
                        for k in range(nck):
                            rows = 128 if k < 4 else 2
                            lhsT = xm[:, k * 512 + yy * 128:
                                      k * 512 + yy * 128 + 128] if k < 4 \
                                else st["xb2"][:, yy * 128:yy * 128 + 128]
                            nc.tensor.matmul(
                                pv[:, q2 * 256:(q2 + 1) * 256], lhsT,
                                WV[0:rows, k * CV:(k + 1) * CV],
                                start=(k == 0), stop=(k == nck - 1))
                    ev(vb_s[:, vsl + g * 2:vsl + g * 2 + 2, 0:256], pv[:])

                # row energies E[i, x] for the 4 rows
                pE = wkp.tile([128, 512], F32, tag="wk")
                for yy in range(4):
                    y = ch * 4 + yy
                    ysl = slice(y * 128, (y + 1) * 128)
                    nc.tensor.matmul(pE[:, yy * 128:(yy + 1) * 128],
                                     K2[:, ysl], QK[0:64, ysl],
                                     start=True, stop=True)
                # batched v write (contiguous 2 KB runs per partition)
                nc.gpsimd.dma_start(vscr_w[:, ch],
                                    vb_s[:, vsl:vsl + 4, 0:256])
                p4 = p4p.tile([128, 512], BF16, tag="p4")
                nc.scalar.activation(p4[:], pE[:], EXP)
                row_state[ch] = p4

            def row_tail(ch):
                vsl = (ch % 2) * 4
                p4 = row_state.pop(ch)
                o16 = o16rp.tile([128, 1032], BF16, tag="o16r")
                for g in range(2):
                    pO = pOp.tile([128, 1024], F32, tag="pO")
                    for q2 in range(2):
                        yy = g * 2 + q2
                        nc.tensor.matmul(
                            pO[:, q2 * 512:q2 * 512 + VBW],
                            p4[:, yy * 128:(yy + 1) * 128],
                            vb_s[:, vsl + yy, :], start=True, stop=True)
                    ev(o16[:, g * 516:(g + 1) * 516],
                       pO[:].rearrange("p (b k) -> p b k", k=512)[:, :, 0:VBW])
                nc.sync.dma_start(orow_v[:, ch], o16[:])

            for ch in range(33):
                if ch < 32:
                    if ch + 4 < 32:
                        load_x(ch + 4)
                    row_head(ch)
                if ch >= 1:
                    row_tail(ch - 1)

            # =================== column pass ===================
            def gather_v(sch):
                vt = (sch % 4) * 8
                nc.gpsimd.dma_start(vtb_s[:, vt:vt + 8, 0:256],
                                    vscr_r[:, sch])

            gather_v(0)
            gather_v(1)

            def col_step(sch):
                if 0 <= sch + 2 < 16:
                    gather_v(sch + 2)
                prev = col_state.pop(sch - 1, None)
                newstate = []
                for g in range(2):
                    pE = None
                    if sch < 16:
                        pE = wkp.tile([128, 512], F32, tag="wk")
                        for xx in range(4):
                            x = sch * 8 + g * 4 + xx
                            nc.tensor.matmul(
                                pE[:, xx * 128:(xx + 1) * 128],
                                K2T[:, x * 128:(x + 1) * 128],
                                qk_of[:, :, x], start=True, stop=True)
                    if prev is not None:
                        vt = ((sch - 1) % 4) * 8
                        o16 = o16cp.tile([128, 1032], BF16, tag="o16c")
                        for half in range(2):
                            pO = pOp.tile([128, 1024], F32, tag="pO")
                            for q2 in range(2):
                                xx = g * 4 + half * 2 + q2
                                nc.tensor.matmul(
                                    pO[:, q2 * 512:q2 * 512 + VBW],
                                    prev[g][:, (half * 2 + q2) * 128:
                                            (half * 2 + q2 + 1) * 128],
                                    vtb_s[:, vt + xx, :],
                                    start=True, stop=True)
                            ev(o16[:, half * 516:(half + 1) * 516],
                               pO[:].rearrange("p (b k) -> p b k",
                                               k=512)[:, :, 0:VBW])
                        nc.sync.dma_start(ocol_v[:, (sch - 1) * 2 + g], o16[:])
                    if sch < 16:
                        p4 = p4p.tile([128, 512], BF16, tag="p4")
                        nc.scalar.activation(p4[:], pE[:], EXP)
                        p4m = p4p.tile([128, 512], BF16, tag="p4m")
                        nc.vector.scalar_tensor_tensor(p4m[:], p4[:], 1.0,
                                                       DMASK[:], MULT, MULT)
                        newstate.append(p4m)
                if sch < 16:
                    col_state[sch] = newstate

            for sch in range(17):
                col_step(sch)

    nc.compile()
    return nc


def _get_nc(with_bias):
    key = bool(with_bias)
    if key not in _CACHE:
        _CACHE[key] = _build(key)
    return _CACHE[key]


def kernel(x, Wq, bq, Wk, bk, Wv, bv, _trace=False):
    import ml_dtypes
    bf = ml_dtypes.bfloat16
    f16 = np.float16

    x = np.asarray(x, np.float32)
    Wq = np.asarray(Wq, np.float32)
    Wk = np.asarray(Wk, np.float32)
    Wv = np.asarray(Wv, np.float32)
    bq = np.asarray(bq, np.float32)
    bk = np.asarray(bk, np.float32)
    bv = np.asarray(bv, np.float32)

    with_bias = bool(np.any(bq) or np.any(bk) or np.any(bv))
    nc = _get_nc(with_bias)

    dmask_a = np.ones((128, 512), np.float32)
    for xx in range(4):
        dmask_a[:, xx * 128:(xx + 1) * 128] -= np.eye(128, dtype=np.float32)
    dmask_a = np.ascontiguousarray(dmask_a.astype(bf))
    ones_a = np.ones((128, 96), bf)

    wqk_full = np.concatenate([Wq.T, Wk.T], axis=1)       # [C, 128]
    if with_bias:
        bias_qk = np.concatenate([bq, bk])[None, :]       # [1, 128]
        wqk_full = np.concatenate(
            [wqk_full, bias_qk, np.zeros_like(bias_qk)], axis=0)
    wqk_full = np.ascontiguousarray(wqk_full.astype(f16))

    in_maps = []
    for core in range(N_CORES):
        b, h = core // 2, core % 2
        xb = x[b].reshape(C, HW)
        wvh = Wv[h * CV:(h + 1) * CV, :].T                # [C, CV]
        if with_bias:
            xb = np.concatenate([xb, np.ones((1, HW), np.float32),
                                 np.zeros((1, HW), np.float32)], axis=0)
            bvh = bv[h * CV:(h + 1) * CV][None, :]
            wvh = np.concatenate([wvh, bvh, np.zeros_like(bvh)], axis=0)
        in_maps.append({
            "xin": np.ascontiguousarray(xb.astype(f16)),
            "wqk": wqk_full,
            "wv": np.ascontiguousarray(wvh.astype(f16)),
            "dmask": dmask_a, "onesc": ones_a,
        })

    res = run_bass_kernel_spmd(nc, in_maps, list(range(N_CORES)),
                               trace=bool(_trace))
    globals()["LAST_RES"] = res

    out = np.empty((B, C, H, W), np.float32)
    for core in range(N_CORES):
        b, h = core // 2, core % 2
        r = res.results[core]
        o_r4 = r["orow"].astype(np.float32).reshape(128, 32, 4, 258)
        o_c4 = r["ocol"].astype(np.float32).reshape(128, 32, 4, 258)
        o_r = o_r4[..., :256].transpose(1, 2, 0, 3).reshape(H, W, CV)  # [y,x,c]
        o_c = o_c4[..., :256].transpose(1, 2, 0, 3).reshape(W, H, CV)  # [x,y,c]
        sr = o_r4[..., 256].reshape(128, 128)            # [x, y]
        sc = o_c4[..., 256].reshape(128, 128)            # [y, x]
        denom = sr.T + sc                                # [y, x]
        comb = (o_r + o_c.transpose(1, 0, 2)) / denom[:, :, None]
        out[b, h * CV:(h + 1) * CV] = comb.transpose(2, 0, 1)

    if _trace:
        return out, res
    return out
